# revision 1
# baseline (speedup 1.0000x reference)
"""DeepseekV3-style SwiGLU MLP with block-dequantized weights on 8 Trainium2
NeuronCores.

Math (per reference):
    wg = gate_weight * blockscale(gate_scale)   # [I, H], 128x128 blocks
    wu = up_weight   * blockscale(up_scale)
    wd = down_weight * blockscale(down_scale)
    gate = x @ wg.T        # [T, I]
    up   = x @ wu.T
    h    = silu(gate) * up
    out  = h @ wd          # [T, H]

Sharding: tensor-parallel over the intermediate dim I across 8 cores
(column-parallel gate/up, row-parallel down), ReduceScatter on the
down-proj partials, host-side concat of the disjoint slices.

Device layout choices (host prepares these in numpy):
  - x is shipped transposed (xt = x.T, [H, T]) so the contraction dim H
    lands on SBUF partitions for the gate/up matmuls.
  - gate/up weights shipped as w^T tiles: wg_prep[ib, p, hb, i] =
    w[ib*128+i, hb*128+p], giving [128, HB*128] SBUF tiles whose hb-slices
    are ready-to-use matmul lhsT ([K=h, M=i]) with contiguous 16KB
    per-partition DMA lines.
  - down weight shipped in native [i, h] orientation, split into NQ
    column-quarters: wd_prep[q, ib, p, j] = w[ib*128+p, q*HQ+j].
  - scales shipped pre-broadcast across partitions: sc[p, ib*HB+hb] =
    scale[ib, hb], so on-chip dequant is one DVE tensor_tensor multiply
    with a stride-0 free-dim broadcast AP per weight tile.

Matmuls run as float32r (fp32 rounded to ~11 mantissa bits by the producing
DVE op — the BIR verifier requires fp32r operands to come from a compute op
with fp32r output, so weights go DMA(raw f32 tile) -> DVE dequant-multiply
into an f32r tile, and x^T goes DMA(raw) -> DVE copy). fp32r streams at
1 cycle/row for moving free-dim >= 256, same as bf16; PSUM accumulates fp32.
Measured end-to-end rel err vs the fp32 reference: ~2.6e-4.

Per-core compute:
  phase 1: for each of IB i-tiles: stream gate/up weight chunks [128, 8, 128]
           (DMA alternating between the two HWDGE rings, SP and ACT), dequant
           on DVE, 2x32 matmuls accumulating over hb into two PSUM banks,
           silu on ACT, silu*up on DVE into a resident h tile [128, IB, 512].
  phase 2: for each output quarter: stream+dequant 11 wd chunks, 8 PSUM
           accumulators ([t 0..3] x [hc 0..1]) accumulated over ib, evacuate
           via ACT, DMA to a DRAM bounce, ReduceScatter across the 8 cores,
           DMA the local 64-row slice to the external output.

Real HW timing (no NTFF profiling available under this axon client): the
no-collective body measures 333-366 us/iteration via an in-NEFF For_i
wall-clock slope (bench_loop.py); the 4 chunked ReduceScatters add ~50 us.
"""

import os

import numpy as np

P = 128
T = 512
H = 4096
I_FULL = 11008
NCORES = 8
IB = 11                 # 128-row i-blocks per core (padded 86 -> 88 blocks)
I_CORE = IB * P         # 1408
I_PAD = NCORES * I_CORE  # 11264
HB = H // P             # 32
NQ = 4                  # down-proj output column quarters
HQ = H // NQ            # 1024
HQB = HQ // P           # 8
TT = T // P             # 4
RS_ROWS = T // NCORES   # 64

LAST_RESULTS = None  # BassKernelResults from the most recent run (for test.py)
_PROG_CACHE = {}     # (mm_dtype, use_rs, merged) -> lowered Bass program


def _build_program(mm_dtype: str, use_rs: bool, loop_n: int = 1,
                   hc: int | None = None, wraw_bufs: int | None = None,
                   wdeq_bufs: int | None = None, phases: str = "12",
                   dma_split: bool = False, merged_gu: bool = False):
    import contextlib

    import concourse.mybir as mybir
    from concourse import bacc
    from concourse.bass import ds, ts
    from concourse.tile import TileContext

    f32 = mybir.dt.float32
    AF = mybir.ActivationFunctionType
    ALU = mybir.AluOpType

    # mm_dt: dtype the matmuls consume (what the dequant DVE op writes).
    # raw_dt: dtype the weights arrive in DRAM.
    if mm_dtype == "bf16":
        mm_dt = mybir.dt.bfloat16
        raw_dt = mybir.dt.bfloat16
    elif mm_dtype == "f32":
        mm_dt = f32
        raw_dt = f32
    else:  # f32r
        mm_dt = mybir.dt.float32r
        raw_dt = f32

    nc = bacc.Bacc("TRN2", num_devices=NCORES)

    xt = nc.dram_tensor("xt", [H, T], raw_dt, kind="ExternalInput")
    NCK8 = HB // 8
    if merged_gu:
        wgu = nc.dram_tensor("wgu", [IB, NCK8, P, 16, P], raw_dt,
                             kind="ExternalInput")
        sgu = nc.dram_tensor("sgu", [P, IB * NCK8 * 16], f32,
                             kind="ExternalInput")
    else:
        wg = nc.dram_tensor("wg", [IB, P, HB, P], raw_dt, kind="ExternalInput")
        wu = nc.dram_tensor("wu", [IB, P, HB, P], raw_dt, kind="ExternalInput")
        sg = nc.dram_tensor("sg", [P, IB * HB], f32, kind="ExternalInput")
        su = nc.dram_tensor("su", [P, IB * HB], f32, kind="ExternalInput")
    wd = nc.dram_tensor("wd", [NQ, IB, P, HQ], raw_dt, kind="ExternalInput")
    sd = nc.dram_tensor("sd", [P, IB * HB], f32, kind="ExternalInput")
    if use_rs:
        out = nc.dram_tensor("out", [NQ, RS_ROWS, HQ], f32, kind="ExternalOutput")
        rs_in = nc.dram_tensor("rs_in", [NQ, T, HQ], f32, kind="Internal")
        rs_out = nc.dram_tensor("rs_out", [NQ, RS_ROWS, HQ], f32, kind="Internal")
    else:
        out = nc.dram_tensor("out", [T, H], f32, kind="ExternalOutput")

    with TileContext(nc) as tc:
        with (
            tc.tile_pool(name="const", bufs=1) as cpool,
            tc.tile_pool(name="wraw",
                         bufs=wraw_bufs or (4 if merged_gu else 6)) as wraw_pool,
            tc.tile_pool(name="wdeq",
                         bufs=wdeq_bufs or (5 if merged_gu else 8)) as wdeq_pool,
            tc.tile_pool(name="silp", bufs=2) as sil_pool,
            tc.tile_pool(name="oevp", bufs=4) as oev_pool,
            tc.tile_pool(name="psum", bufs=8, space="PSUM") as ps_pool,
        ):
            # Timing mode: run the whole body loop_n times inside the NEFF so
            # per-iteration HW time can be read off the wall-clock slope.
            assert loop_n == 1 or not use_rs, "collectives can't sit in For_i"
            loop_cm = (
                tc.For_i(0, loop_n, 1) if loop_n > 1 else contextlib.nullcontext()
            )
            loop_cm.__enter__()
            # Resident tiles: x^T, the three broadcast scale rows, and the
            # SwiGLU intermediate h^T (written phase 1, read phase 2).
            xt_sb = cpool.tile([P, HB, T], mm_dt)
            xt_ap = xt.rearrange("(hb p) t -> p hb t", p=P)
            if mm_dtype == "f32r":
                # DMA can't produce "rounded to FP32r" data for the verifier;
                # stage through raw chunk tiles and round via a DVE copy.
                with tc.tile_pool(name="xraw", bufs=2) as xraw_pool:
                    for xc in range(8):
                        xr = xraw_pool.tile([P, 4, T], f32, tag="xr",
                                            name=f"xr{xc}")
                        nc.sync.dma_start(xr[:], xt_ap[:, ds(xc * 4, 4), :])
                        nc.vector.tensor_copy(
                            xt_sb[:, ds(xc * 4, 4), :], xr[:]
                        )
            else:
                for xc in range(4):
                    nc.sync.dma_start(
                        xt_sb[:, ds(xc * 8, 8), :], xt_ap[:, ds(xc * 8, 8), :]
                    )
            if merged_gu:
                sgu_sb = cpool.tile([P, IB * NCK8 * 16], f32)
                nc.sync.dma_start(sgu_sb[:], sgu[:])
            else:
                sc_sb = cpool.tile([P, 2, IB * HB], f32)
                for j, s in enumerate((sg, su)):
                    nc.sync.dma_start(sc_sb[:, j, :], s[:])
            sd_sb = cpool.tile([P, IB * HB], f32)
            nc.sync.dma_start(sd_sb[:], sd[:])
            h_all = cpool.tile([P, IB, T], mm_dt)

            HC = hc or 8  # hb-chunk width for streamed weight tiles [P, HC, P]

            dma_engines = [nc.sync, nc.scalar] if dma_split else [nc.sync]
            dma_rr = [0]

            MX = max(HC, HQB, 16 if merged_gu else 0)

            def load_deq(dram_ap, scale_row, name, chunk=None):
                """DMA a [P, chunk, P] raw weight chunk, dequant+round into
                an mm_dt tile via one DVE tensor_tensor with the [P, chunk]
                scale_row broadcast along the inner 128, return the deq tile.
                Tiles are allocated at the max chunk size so all weight
                streams share the same two pool tags."""
                ck = chunk or HC
                raw = wraw_pool.tile([P, MX, P], raw_dt, tag="wraw",
                                     name=f"raw_{name}")[:, :ck, :]
                eng = dma_engines[dma_rr[0] % len(dma_engines)]
                dma_rr[0] += 1
                eng.dma_start(raw, dram_ap)
                deq = wdeq_pool.tile([P, MX, P], mm_dt, tag="wdeq",
                                     name=f"deq_{name}")[:, :ck, :]
                nc.vector.tensor_tensor(
                    deq,
                    raw,
                    scale_row[:, :, None].to_broadcast([P, ck, P]),
                    ALU.mult,
                )
                return deq

            # ---- phase 1: gate/up projections + SwiGLU --------------------
            NCK = HB // HC  # chunks per i-tile
            for ib in range(IB if "1" in phases else 0):
                ps_g = ps_pool.tile([P, T], f32, tag="ps")
                ps_u = ps_pool.tile([P, T], f32, tag="ps")
                for c in range(NCK):
                    if merged_gu:
                        guq = load_deq(
                            wgu[ib, c],
                            sgu_sb[:, ds((ib * NCK8 + c) * 16, 16)],
                            f"gu{ib}_{c}", chunk=16,
                        )
                        gq, uq = guq[:, :HC, :], guq[:, HC:, :]
                    else:
                        gq = load_deq(wg[ib, :, ds(c * HC, HC), :],
                                      sc_sb[:, 0, ds(ib * HB + c * HC, HC)],
                                      f"g{ib}_{c}", chunk=HC)
                        uq = load_deq(wu[ib, :, ds(c * HC, HC), :],
                                      sc_sb[:, 1, ds(ib * HB + c * HC, HC)],
                                      f"u{ib}_{c}", chunk=HC)
                    for off in range(HC):
                        hb = c * HC + off
                        nc.tensor.matmul(
                            ps_g[:], gq[:, off], xt_sb[:, hb],
                            start=(hb == 0), stop=(hb == HB - 1),
                        )
                        nc.tensor.matmul(
                            ps_u[:], uq[:, off], xt_sb[:, hb],
                            start=(hb == 0), stop=(hb == HB - 1),
                        )
                sil = sil_pool.tile([P, T], f32, tag="sil")
                nc.scalar.activation(sil[:], ps_g[:], AF.Silu)
                nc.vector.tensor_tensor(h_all[:, ib, :], sil[:], ps_u[:], ALU.mult)

            # ---- phase 2: down projection + ReduceScatter -----------------
            for q in range(NQ if "2" in phases else 0):
                ps_o = [
                    ps_pool.tile([P, 512], f32, tag="ps", name=f"ps_o_{q}_{i}")
                    for i in range(TT * 2)
                ]
                for ib in range(IB):
                    dq = load_deq(
                        wd[q, ib].rearrange("p (b j) -> p b j", j=P),
                        sd_sb[:, ds(ib * HB + q * HQB, HQB)],
                        f"d{q}_{ib}", chunk=HQB,
                    )
                    for t in range(TT):
                        for hcc in range(2):
                            nc.tensor.matmul(
                                ps_o[t * 2 + hcc][:],
                                h_all[:, ib, ts(t, P)],
                                dq[:, ds(hcc * 4, 4)],
                                start=(ib == 0),
                                stop=(ib == IB - 1),
                            )
                for t in range(TT):
                    for hc in range(2):
                        ot = oev_pool.tile([P, 512], f32, tag="oev",
                                           name=f"ot_{q}_{t}_{hc}")
                        # ACT copy. A DVE tensor_copy here measured ~7 us
                        # faster in the cost model but hit
                        # NRT_EXEC_UNIT_UNRECOVERABLE on hardware; ACT is the
                        # verified-stable evacuation path.
                        nc.scalar.copy(ot[:], ps_o[t * 2 + hc][:])
                        if use_rs:
                            dst = rs_in[q, ds(t * P, P), ds(hc * 512, 512)]
                        else:
                            dst = out[ds(t * P, P), ds(q * HQ + hc * 512, 512)]
                        nc.sync.dma_start(dst, ot[:])
                if use_rs:
                    nc.gpsimd.collective_compute(
                        "ReduceScatter",
                        ALU.add,
                        replica_groups=[list(range(NCORES))],
                        ins=[rs_in[q]],
                        outs=[rs_out[q]],
                    )
                    nc.sync.dma_start(out[q], rs_out[q])

            loop_cm.__exit__(None, None, None)

    nc.compile()  # bacc lowering: register alloc + multi-wait splitting
    return nc


def _prep_inputs(x, gate_weight, up_weight, down_weight, gate_scale, up_scale,
                 down_scale, mm_dtype, merged_gu=False):
    """Pad/shard/transpose on the host into the per-core DMA-friendly layouts."""
    if mm_dtype == "bf16":
        import ml_dtypes

        w_np = ml_dtypes.bfloat16
    else:
        w_np = np.float32

    x = np.asarray(x, np.float32)
    xt = x.T.astype(w_np) if w_np != np.float32 else np.ascontiguousarray(x.T)

    def pad_w(w):
        wp = np.zeros((I_PAD, H), np.float32)
        wp[:I_FULL] = np.asarray(w, np.float32)
        return wp

    def pad_s(s):
        sp = np.zeros((I_PAD // P, HB), np.float32)
        sp[: I_FULL // P] = np.asarray(s, np.float32)
        return sp

    gw, uw, dw = pad_w(gate_weight), pad_w(up_weight), pad_w(down_weight)
    gs, us, dsc = pad_s(gate_scale), pad_s(up_scale), pad_s(down_scale)

    in_maps = []
    for c in range(NCORES):
        i0 = c * I_CORE
        gwc = gw[i0 : i0 + I_CORE]
        uwc = uw[i0 : i0 + I_CORE]
        dwc = dw[i0 : i0 + I_CORE]
        # gate/up: [ib, p(h-in-block), hb, i] = w[ib*128+i, hb*128+p]
        def _c(a):
            return a.astype(w_np) if a.dtype != w_np else np.ascontiguousarray(a)

        wg_prep = _c(gwc.reshape(IB, P, HB, P).transpose(0, 3, 2, 1))
        wu_prep = _c(uwc.reshape(IB, P, HB, P).transpose(0, 3, 2, 1))
        # down: [q, ib, p(i-in-block), j(h-in-quarter)] = w[ib*128+p, q*HQ+j]
        wd_prep = _c(dwc.reshape(IB, P, NQ, HQ).transpose(2, 0, 1, 3))

        def bscale(s):
            row = np.ascontiguousarray(s[c * IB : (c + 1) * IB]).reshape(1, IB * HB)
            return np.ascontiguousarray(np.broadcast_to(row, (P, IB * HB))).astype(
                np.float32
            )

        if merged_gu:
            NCK8, HC8 = HB // 8, 8
            g5 = wg_prep.reshape(IB, P, NCK8, HC8, P).transpose(0, 2, 1, 3, 4)
            u5 = wu_prep.reshape(IB, P, NCK8, HC8, P).transpose(0, 2, 1, 3, 4)
            wgu_prep = np.ascontiguousarray(
                np.stack([g5, u5], axis=3).reshape(IB, NCK8, P, 2 * HC8, P)
            )
            sgb = bscale(gs).reshape(P, IB, NCK8, HC8)
            sub = bscale(us).reshape(P, IB, NCK8, HC8)
            sgu_prep = np.ascontiguousarray(
                np.stack([sgb, sub], axis=3).reshape(P, IB * NCK8 * 2 * HC8)
            )
            in_maps.append(
                {
                    "xt": xt,
                    "wgu": wgu_prep,
                    "wd": wd_prep,
                    "sgu": sgu_prep,
                    "sd": bscale(dsc),
                }
            )
        else:
            in_maps.append(
                {
                    "xt": xt,
                    "wg": wg_prep,
                    "wu": wu_prep,
                    "wd": wd_prep,
                    "sg": bscale(gs),
                    "su": bscale(us),
                    "sd": bscale(dsc),
                }
            )
    return in_maps


def kernel(x, gate_weight, up_weight, down_weight, gate_scale, up_scale,
           down_scale, blocksize):
    global LAST_RESULTS
    assert int(blocksize) == P, f"kernel hardcodes blocksize=128, got {blocksize}"

    from concourse.bass_utils import run_bass_kernel_spmd

    mm_dtype = os.environ.get("MLP_MM_DTYPE", "f32r")
    use_rs = os.environ.get("MLP_USE_RS", "1") == "1"
    trace = os.environ.get("BASS_TRACE", "0") == "1"
    merged = os.environ.get("MLP_MERGED", "0") == "1"

    key = (mm_dtype, use_rs, merged)
    nc = _PROG_CACHE.get(key)
    if nc is None:
        nc = _build_program(mm_dtype, use_rs, dma_split=True, merged_gu=merged)
        _PROG_CACHE[key] = nc
    in_maps = _prep_inputs(
        x, gate_weight, up_weight, down_weight, gate_scale, up_scale, down_scale,
        mm_dtype, merged_gu=merged,
    )
    results = run_bass_kernel_spmd(
        nc, in_maps, core_ids=list(range(NCORES)), trace=trace
    )
    LAST_RESULTS = results

    full = np.empty((T, H), np.float32)
    if use_rs:
        for c, res in enumerate(results.results):
            o = res["out"]  # [NQ, RS_ROWS, HQ]
            for q in range(NQ):
                full[c * RS_ROWS : (c + 1) * RS_ROWS, q * HQ : (q + 1) * HQ] = o[q]
    else:
        acc = np.zeros((T, H), np.float64)
        for res in results.results:
            acc += res["out"]
        full = acc.astype(np.float32)
    return full

